# revision 2
# baseline (speedup 1.0000x reference)
"""TT-matrix dense layer (KerasDense via tensor-train) on 8 TRN2 NeuronCores.

out[b, :] = relu(x[b, :] @ W + bias),  W = TT(core0..core3), 4096x4096.

Cores merged pairwise:
  W01[(m1 m2), (n1 n2), s2] = sum_s1 core0[0,m1,n1,s1] * core1[s1,m2,n2,s2]
  W23[s2, (m3 m4), (n3 n4)] = sum_s3 core2[s2,m3,n3,s3] * core3[s3,m4,n4,0]
  out[b, (n12 n34)] = relu( sum_{s2,m12,m34}
        x[b, (m12 m34)] * W01[m12, n12, s2] * W23[s2, m34, n34] + bias )

Per-core dataflow (batch sharded 8 ways, 2048 rows/core):
  xT2 [128, 128, 64]: partition = m12 (two 64-row b-groups), free = (b, m34),
      loaded straight from DRAM with 256B-contiguous runs.
  step1 (contract m12): per b: stationary X_b [m12, m34], rhs W01r [m12, (s2 n12)]
      -> v_b [m34, (s2 n12)] in PSUM; 4 b's packed via row/col tile_position.
  step2 (contract m34, PSUM-accumulate s2): stationary W23r_s2 [m34, n34],
      rhs v [m34, (slot n12)] -> out.T blocks [n34, (slot n12)].
  bias+relu, PE-transpose back to b-major, batched DMA out (256B runs).
"""

import numpy as np

BATCH = 16384
NCORES = 8
B = BATCH // NCORES     # 2048 rows per core
F = 4096
R = 8                   # TT bond s2
SUP = 256               # rows per x-load super-tile
NSUP = B // SUP         # 8
CH_ROUNDS = 8           # 4-b rounds per chunk
NCH = 8                 # chunks per super-tile (64 rounds total)
SLOTS = CH_ROUNDS

_compiled = None


def _build():
    from contextlib import ExitStack
    from concourse import bacc, tile, mybir, masks

    dt = mybir.dt.float32
    nc = bacc.Bacc("TRN2", target_bir_lowering=False, debug=False)

    x_d = nc.dram_tensor("x", [B, F], dt, kind="ExternalInput")
    w01_d = nc.dram_tensor("w01", [128, 512], dt, kind="ExternalInput")
    w23_d = nc.dram_tensor("w23", [128, R, 64], dt, kind="ExternalInput")
    biasq_d = nc.dram_tensor("biasq", [128, SLOTS, 64], dt, kind="ExternalInput")
    out_d = nc.dram_tensor("out", [B, F], dt, kind="ExternalOutput")

    with tile.TileContext(nc) as tc, ExitStack() as ctx:
        const = ctx.enter_context(tc.tile_pool(name="const", bufs=1))
        xpool = ctx.enter_context(tc.tile_pool(name="x", bufs=2))
        vpool = ctx.enter_context(tc.tile_pool(name="v", bufs=2))
        opool = ctx.enter_context(tc.tile_pool(name="o", bufs=2))
        ps1 = ctx.enter_context(tc.tile_pool(name="ps1", bufs=2, space="PSUM"))
        ps2 = ctx.enter_context(tc.tile_pool(name="ps2", bufs=1, space="PSUM"))
        ps3 = ctx.enter_context(tc.tile_pool(name="ps3", bufs=2, space="PSUM"))

        w01 = const.tile([128, 512], dt)
        w23 = const.tile([128, R, 64], dt)
        biasq = const.tile([128, SLOTS, 64], dt)
        ident = const.tile([128, 128], dt)
        nc.sync.dma_start(w01[:], w01_d.ap())
        nc.sync.dma_start(w23[:], w23_d.ap())
        nc.sync.dma_start(biasq[:], biasq_d.ap())
        masks.make_identity(nc, ident[:])

        # x DRAM view: [half g, m12, b_local, m34]; 256B runs (m34 contiguous)
        xv = x_d.ap().rearrange("(g b) (m1 m34) -> g m1 b m34", g=NSUP * 2, m1=64)
        out_v = out_d.ap().rearrange("b (n12 n34) -> b n12 n34", n34=64)

        for st in range(NSUP):
            xT2 = xpool.tile([128, 128, 64], dt, tag="xT2")
            nc.sync.dma_start(xT2[0:64], xv[2 * st])
            nc.sync.dma_start(xT2[64:128], xv[2 * st + 1])

            for ch in range(NCH):
                vE = vpool.tile([128, SLOTS, 512], dt, tag="vE")
                vO = vpool.tile([128, SLOTS, 512], dt, tag="vO")
                # ---- step 1: per-b stationary matmuls, 4-packed ----
                for r in range(CH_ROUNDS):
                    j = ch * 2 * CH_ROUNDS + 2 * r  # b_local pair (j, j+1)
                    pU0 = ps1.tile([128, 512], dt, tag="U0")
                    pU1 = ps1.tile([128, 512], dt, tag="U1")
                    nc.tensor.matmul(pU0[0:64], xT2[0:64, j], w01[0:64],
                                     start=True, stop=True, tile_position=(0, 0))
                    nc.tensor.matmul(pU0[64:128], xT2[0:64, j + 1], w01[0:64],
                                     start=True, stop=True, tile_position=(0, 64))
                    nc.tensor.matmul(pU1[0:64], xT2[64:128, j], w01[64:128],
                                     start=True, stop=True, tile_position=(64, 0))
                    nc.tensor.matmul(pU1[64:128], xT2[64:128, j + 1], w01[64:128],
                                     start=True, stop=True, tile_position=(64, 64))
                    nc.vector.tensor_copy(vE[:, r], pU0[:])
                    nc.scalar.copy(vO[:, r], pU1[:])

                # ---- step 2 + epilogue, for each half (E: rows 0-127 of
                # super-tile, O: rows 128-255) ----
                for name, v in (("E", vE), ("O", vO)):
                    po = ps2.tile([128, SLOTS, 64], dt, tag="po" + name)
                    v4 = v[:].rearrange("p t (s n) -> p t s n", s=R)
                    for s2 in range(R):
                        nc.tensor.matmul(po[0:64], w23[0:64, s2],
                                         v4[0:64, :, s2],
                                         start=(s2 == 0), stop=(s2 == R - 1),
                                         tile_position=(0, 0))
                        nc.tensor.matmul(po[64:128], w23[64:128, s2],
                                         v4[64:128, :, s2],
                                         start=(s2 == 0), stop=(s2 == R - 1),
                                         tile_position=(64, 64))
                    # bias add (relu commutes with the transpose; applied after)
                    ot = opool.tile([128, SLOTS, 64], dt, tag="ot" + name)
                    nc.vector.tensor_tensor(ot[:], po[:], biasq[:],
                                            mybir.AluOpType.add)
                    # ---- transpose back to b-major ----
                    # thin transposes per b-parity: in [64, 128] -> out
                    # [128=(sp,n12), 64=n34]; b = b0 + 4t + 2sp + par
                    o2 = ot[:].rearrange("p t n -> p (t n)")
                    b0 = st * SUP + (128 if name == "O" else 0) + 16 * ch
                    dv = out_v[b0:b0 + 16].rearrange(
                        "(t sp par) n12 n34 -> par sp n12 t n34", t=4, sp=2)
                    for par in range(2):
                        pt = ps3.tile([128, 4, 64], dt, tag="pt")
                        for t in range(4):
                            nc.tensor.transpose(
                                pt[:, t],
                                o2[64 * par:64 * (par + 1),
                                   128 * t:128 * (t + 1)],
                                ident[64 * par:64 * (par + 1),
                                      64 * par:64 * (par + 1)])
                        ob = opool.tile([128, 4, 64], dt, tag="ob" + name)
                        nc.scalar.activation(ob[:], pt[:],
                                             mybir.ActivationFunctionType.Relu)
                        nc.sync.dma_start(dv[par, 0], ob[0:64])
                        nc.sync.dma_start(dv[par, 1], ob[64:128])

    nc.compile()
    return nc


def _prep_weights(core0, core1, core2, core3, bias):
    # w01[(m1,m2),(n1,n2),s2]; einsum dims: m=m1 n=n1 s=s1 / o=m2 p=n2 r=s2
    w01 = np.einsum("mns,sopr->monpr", core0[0], core1).reshape(64, 64, 8)
    # w23[s2,(m3,m4),(n3,n4)]; s m=m3 n=n3 t=s3 / p=m4 q=n4
    w23 = np.einsum("smnt,tpq->smpnq", core2, core3[:, :, :, 0]).reshape(8, 64, 64)

    w01r = w01.transpose(0, 2, 1).reshape(64, 512)        # [m12, (s2 n12)]
    w01r = np.concatenate([w01r, w01r], 0).astype(np.float32)
    w23r = w23.transpose(1, 0, 2)                          # [m34, s2, n34]
    w23r = np.concatenate([w23r, w23r], 0).astype(np.float32)
    bq = bias.reshape(64, 64).T                            # [n34, n12]
    bq = np.broadcast_to(bq[None, :, None, :], (2, 64, SLOTS, 64))
    biasq = np.ascontiguousarray(bq.reshape(128, SLOTS, 64), np.float32)
    return w01r, w23r, biasq


def kernel(x, core0, core1, core2, core3, bias):
    global _compiled
    from concourse.bass_utils import run_bass_kernel_spmd

    if _compiled is None:
        _compiled = _build()
    nc = _compiled

    w01r, w23r, biasq = _prep_weights(
        np.asarray(core0, np.float32), np.asarray(core1, np.float32),
        np.asarray(core2, np.float32), np.asarray(core3, np.float32),
        np.asarray(bias, np.float32))

    x = np.ascontiguousarray(np.asarray(x, np.float32))
    in_maps = [{
        "x": x[c * B:(c + 1) * B],
        "w01": w01r, "w23": w23r, "biasq": biasq,
    } for c in range(NCORES)]
    res = run_bass_kernel_spmd(nc, in_maps, list(range(NCORES)))
    globals()["_last_results"] = res
    out = np.concatenate([res.results[c]["out"] for c in range(NCORES)], axis=0)
    return out.astype(np.float32)



# revision 8
# speedup vs baseline: 1.9575x; 1.9575x over previous
"""TT-matrix dense layer (KerasDense via tensor-train) on 8 TRN2 NeuronCores.

out[b, :] = relu(x[b, :] @ W + bias),  W = TT(core0..core3), 4096x4096.

Cores merged pairwise:
  W01[(m1 m2), (n1 n2), s2] = sum_s1 core0[0,m1,n1,s1] * core1[s1,m2,n2,s2]
  W23[s2, (m3 m4), (n3 n4)] = sum_s3 core2[s2,m3,n3,s3] * core3[s3,m4,n4,0]
  out[b, (n12 n34)] = relu( sum_{s2,m12,m34}
        x[b, (m12 m34)] * W01[m12, n12, s2] * W23[s2, m34, n34] + bias )

Per-core dataflow (batch sharded 8 ways, 2048 rows/core):
  xT2 [128, 128, 64]: partition = m12 (two 64-row b-groups), free = (b, m34),
      loaded straight from DRAM with 256B-contiguous runs.
  step1 (contract m12): per b: stationary X_b [m12, m34], rhs W01r [m12, (s2 n12)]
      -> v_b [m34, (s2 n12)] in PSUM; 4 b's packed via row/col tile_position.
  step2 (contract m34, PSUM-accumulate s2): stationary W23r_s2 [m34, n34],
      rhs v [m34, (slot n12)] -> out.T blocks [n34, (slot n12)].
  bias+relu, PE-transpose back to b-major, batched DMA out (256B runs).
"""

import numpy as np

BATCH = 16384
NCORES = 8
B = BATCH // NCORES     # 2048 rows per core
F = 4096
R = 8                   # TT bond s2
SUP = 256               # rows per x-load super-tile
NSUP = B // SUP         # 8
CH_ROUNDS = 8           # 4-b rounds per chunk
NCH = 8                 # chunks per super-tile (64 rounds total)
SLOTS = CH_ROUNDS

_compiled = None


def _build():
    from contextlib import ExitStack
    from concourse import bacc, tile, mybir, masks

    dt = mybir.dt.float32
    db = mybir.dt.bfloat16
    nc = bacc.Bacc("TRN2", target_bir_lowering=False, debug=False)

    x_d = nc.dram_tensor("x", [B, F], dt, kind="ExternalInput")
    w01_d = nc.dram_tensor("w01", [128, 512], db, kind="ExternalInput")
    w23_d = nc.dram_tensor("w23", [128, R, 64], db, kind="ExternalInput")
    biasq_d = nc.dram_tensor("biasq", [128, SLOTS, 64], dt, kind="ExternalInput")
    out_d = nc.dram_tensor("out", [B, F], dt, kind="ExternalOutput")

    with tile.TileContext(nc) as tc, ExitStack() as ctx:
        const = ctx.enter_context(tc.tile_pool(name="const", bufs=1))
        xpool = ctx.enter_context(tc.tile_pool(name="x", bufs=2))
        vpool = ctx.enter_context(tc.tile_pool(name="v", bufs=2))
        opool = ctx.enter_context(tc.tile_pool(name="o", bufs=2))
        ps1 = ctx.enter_context(tc.tile_pool(name="ps1", bufs=2, space="PSUM"))
        ps2 = ctx.enter_context(tc.tile_pool(name="ps2", bufs=1, space="PSUM"))
        ps3 = ctx.enter_context(tc.tile_pool(name="ps3", bufs=2, space="PSUM"))

        w01 = const.tile([128, 512], db)
        w23 = const.tile([128, R, 64], db)
        biasq = const.tile([128, SLOTS, 64], dt)
        ident = const.tile([128, 128], db)
        nc.sync.dma_start(w01[:], w01_d.ap())
        nc.sync.dma_start(w23[:], w23_d.ap())
        nc.sync.dma_start(biasq[:], biasq_d.ap())
        masks.make_identity(nc, ident[:])

        # x DRAM view: [half g, m12, b_local, m34]; 256B runs (m34 contiguous)
        xv = x_d.ap().rearrange("(g b) (m1 m34) -> g m1 b m34", g=NSUP * 2, m1=64)
        out_v = out_d.ap().rearrange("b (n12 n34) -> b n12 n34", n34=64)

        for st in range(NSUP):
            xT2 = xpool.tile([128, 128, 64], dt, tag="xT2")
            nc.sync.dma_start(xT2[0:64], xv[2 * st])
            nc.sync.dma_start(xT2[64:128], xv[2 * st + 1])
            # convert the super-tile to bf16 on gpsimd (matmul inputs)
            xb = xpool.tile([128, 128, 64], db, tag="xb")
            nc.gpsimd.tensor_copy(xb[:], xT2[:])

            for ch in range(NCH):
                vE = vpool.tile([128, SLOTS, 512], db, tag="vE")
                vO = vpool.tile([128, SLOTS, 512], db, tag="vO")
                # ---- step 1: per-b stationary matmuls, 4-packed ----
                for r in range(CH_ROUNDS):
                    j = ch * 2 * CH_ROUNDS + 2 * r  # b_local pair (j, j+1)
                    pU0 = ps1.tile([128, 512], dt, tag="U0")
                    pU1 = ps1.tile([128, 512], dt, tag="U1")
                    nc.tensor.matmul(pU0[0:64], xb[0:64, j], w01[0:64],
                                     start=True, stop=True, tile_position=(0, 0))
                    nc.tensor.matmul(pU0[64:128], xb[0:64, j + 1], w01[0:64],
                                     start=True, stop=True, tile_position=(0, 64))
                    nc.tensor.matmul(pU1[0:64], xb[64:128, j], w01[64:128],
                                     start=True, stop=True, tile_position=(64, 0))
                    nc.tensor.matmul(pU1[64:128], xb[64:128, j + 1], w01[64:128],
                                     start=True, stop=True, tile_position=(64, 64))
                    nc.vector.tensor_copy(vE[:, r], pU0[:])
                    nc.scalar.copy(vO[:, r], pU1[:])

                # ---- step 2 + epilogue, for each half (E: rows 0-127 of
                # super-tile, O: rows 128-255) ----
                for name, v in (("E", vE), ("O", vO)):
                    po = ps2.tile([128, SLOTS, 64], dt, tag="po" + name)
                    v4 = v[:].rearrange("p t (s n) -> p t s n", s=R)
                    for s2 in range(R):
                        nc.tensor.matmul(po[0:64], w23[0:64, s2],
                                         v4[0:64, :, s2],
                                         start=(s2 == 0), stop=(s2 == R - 1),
                                         tile_position=(0, 0))
                        nc.tensor.matmul(po[64:128], w23[64:128, s2],
                                         v4[64:128, :, s2],
                                         start=(s2 == 0), stop=(s2 == R - 1),
                                         tile_position=(64, 64))
                    # bias add (relu commutes with the transpose; applied after)
                    ot = opool.tile([128, SLOTS, 64], db, tag="ot" + name)
                    nc.vector.tensor_tensor(ot[:], po[:], biasq[:],
                                            mybir.AluOpType.add)
                    # ---- transpose back to b-major ----
                    # thin transposes per b-parity: in [64, 128] -> out
                    # [128=(sp,n12), 64=n34]; b = b0 + 4t + 2sp + par
                    o2 = ot[:].rearrange("p t n -> p (t n)")
                    b0 = st * SUP + (128 if name == "O" else 0) + 16 * ch
                    dv = out_v[b0:b0 + 16].rearrange(
                        "(t sp par) n12 n34 -> par sp n12 t n34", t=4, sp=2)
                    for par in range(2):
                        pt = ps3.tile([128, 4, 64], db, tag="pt")
                        for t in range(4):
                            nc.tensor.transpose(
                                pt[:, t],
                                o2[64 * par:64 * (par + 1),
                                   128 * t:128 * (t + 1)],
                                ident[64 * par:64 * (par + 1),
                                      64 * par:64 * (par + 1)])
                        ob = opool.tile([128, 4, 64], dt, tag="ob" + name)
                        nc.scalar.activation(ob[:], pt[:],
                                             mybir.ActivationFunctionType.Relu)
                        nc.sync.dma_start(dv[par, 0], ob[0:64])
                        nc.sync.dma_start(dv[par, 1], ob[64:128])

    nc.compile()
    return nc


def _prep_weights(core0, core1, core2, core3, bias):
    import ml_dtypes

    # w01[(m1,m2),(n1,n2),s2]; einsum dims: m=m1 n=n1 s=s1 / o=m2 p=n2 r=s2
    w01 = np.einsum("mns,sopr->monpr", core0[0], core1).reshape(64, 64, 8)
    # w23[s2,(m3,m4),(n3,n4)]; s m=m3 n=n3 t=s3 / p=m4 q=n4
    w23 = np.einsum("smnt,tpq->smpnq", core2, core3[:, :, :, 0]).reshape(8, 64, 64)

    w01r = w01.transpose(0, 2, 1).reshape(64, 512)        # [m12, (s2 n12)]
    w01r = np.concatenate([w01r, w01r], 0).astype(ml_dtypes.bfloat16)
    w23r = w23.transpose(1, 0, 2)                          # [m34, s2, n34]
    w23r = np.concatenate([w23r, w23r], 0).astype(ml_dtypes.bfloat16)
    bq = bias.reshape(64, 64).T                            # [n34, n12]
    bq = np.broadcast_to(bq[None, :, None, :], (2, 64, SLOTS, 64))
    biasq = np.ascontiguousarray(bq.reshape(128, SLOTS, 64), np.float32)
    return w01r, w23r, biasq


def kernel(x, core0, core1, core2, core3, bias):
    global _compiled
    from concourse.bass_utils import run_bass_kernel_spmd

    if _compiled is None:
        _compiled = _build()
    nc = _compiled

    w01r, w23r, biasq = _prep_weights(
        np.asarray(core0, np.float32), np.asarray(core1, np.float32),
        np.asarray(core2, np.float32), np.asarray(core3, np.float32),
        np.asarray(bias, np.float32))

    x = np.ascontiguousarray(np.asarray(x, np.float32))
    in_maps = [{
        "x": x[c * B:(c + 1) * B],
        "w01": w01r, "w23": w23r, "biasq": biasq,
    } for c in range(NCORES)]
    res = run_bass_kernel_spmd(nc, in_maps, list(range(NCORES)))
    globals()["_last_results"] = res
    out = np.concatenate([res.results[c]["out"] for c in range(NCORES)], axis=0)
    return out.astype(np.float32)



# revision 18
# speedup vs baseline: 2.0023x; 1.0229x over previous
"""TT-matrix dense layer (KerasDense via tensor-train) on 8 TRN2 NeuronCores.

out[b, :] = relu(x[b, :] @ W + bias),  W = TT(core0..core3), 4096x4096.

Cores merged pairwise:
  W01[(m1 m2), (n1 n2), s2] = sum_s1 core0[0,m1,n1,s1] * core1[s1,m2,n2,s2]
  W23[s2, (m3 m4), (n3 n4)] = sum_s3 core2[s2,m3,n3,s3] * core3[s3,m4,n4,0]
  out[b, (n12 n34)] = relu( sum_{s2,m12,m34}
        x[b, (m12 m34)] * W01[m12, n12, s2] * W23[s2, m34, n34] + bias )

Per-core dataflow (batch sharded 8 ways, 2048 rows/core):
  xT2 [128, 128, 64]: partition = m12 (two 64-row b-groups), free = (b, m34),
      loaded straight from DRAM with 256B-contiguous runs.
  step1 (contract m12): per b: stationary X_b [m12, m34], rhs W01r [m12, (s2 n12)]
      -> v_b [m34, (s2 n12)] in PSUM; 4 b's packed via row/col tile_position.
  step2 (contract m34, PSUM-accumulate s2): stationary W23r_s2 [m34, n34],
      rhs v [m34, (slot n12)] -> out.T blocks [n34, (slot n12)].
  bias+relu, PE-transpose back to b-major, batched DMA out (256B runs).
"""

import numpy as np

BATCH = 16384
NCORES = 8
B = BATCH // NCORES     # 2048 rows per core
F = 4096
R = 8                   # TT bond s2
SUP = 256               # rows per x-load super-tile
NSUP = B // SUP         # 8
CH_ROUNDS = 8           # 4-b rounds per chunk
NCH = 8                 # chunks per super-tile (64 rounds total)
SLOTS = CH_ROUNDS

_compiled = None


def _build():
    from contextlib import ExitStack
    from concourse import bacc, tile, mybir, masks

    dt = mybir.dt.float32
    db = mybir.dt.bfloat16
    nc = bacc.Bacc("TRN2", target_bir_lowering=False, debug=False)

    x_d = nc.dram_tensor("x", [B, F], dt, kind="ExternalInput")
    w01_d = nc.dram_tensor("w01", [128, 512], db, kind="ExternalInput")
    w23_d = nc.dram_tensor("w23", [128, R, 64], db, kind="ExternalInput")
    biasq_d = nc.dram_tensor("biasq", [128, SLOTS, 64], dt, kind="ExternalInput")
    out_d = nc.dram_tensor("out", [B, F], dt, kind="ExternalOutput")

    with tile.TileContext(nc) as tc, ExitStack() as ctx:
        const = ctx.enter_context(tc.tile_pool(name="const", bufs=1))
        xpool = ctx.enter_context(tc.tile_pool(name="x", bufs=2))
        vpool = ctx.enter_context(tc.tile_pool(name="v", bufs=2))
        opool = ctx.enter_context(tc.tile_pool(name="o", bufs=2))
        ps1 = ctx.enter_context(tc.tile_pool(name="ps1", bufs=2, space="PSUM"))
        ps2 = ctx.enter_context(tc.tile_pool(name="ps2", bufs=1, space="PSUM"))
        ps3 = ctx.enter_context(tc.tile_pool(name="ps3", bufs=2, space="PSUM"))

        w01 = const.tile([128, 512], db)
        w23 = const.tile([128, R, 64], db)
        biasq = const.tile([128, SLOTS, 64], dt)
        ident = const.tile([128, 128], db)
        nc.sync.dma_start(w01[:], w01_d.ap())
        nc.sync.dma_start(w23[:], w23_d.ap())
        nc.sync.dma_start(biasq[:], biasq_d.ap())
        masks.make_identity(nc, ident[:])

        # x DRAM view: [half g, m12, b_local, m34]; 256B runs (m34 contiguous)
        xv = x_d.ap().rearrange("(g b) (m1 m34) -> g m1 b m34", g=NSUP * 2, m1=64)
        # out DRAM view for the widened epilogue: b = q*16 + 8*bl + 2*c + tp
        # per (q, tp) the (bl, c) dims merge -> 3-dim DMA APs
        out_v4 = out_d.ap().rearrange(
            "(q bl c tp) (n12 n34) -> q tp n12 bl c n34", bl=2, c=4, tp=2, n34=64)

        for st in range(NSUP):
            xT2 = xpool.tile([128, 128, 64], dt, tag="xT2")
            nc.sync.dma_start(xT2[0:64], xv[2 * st])
            nc.sync.dma_start(xT2[64:128], xv[2 * st + 1])
            # convert the super-tile to bf16 on gpsimd (matmul inputs),
            # reordering b = ch*16 + 8*bl + r so each slot's (bl=0, bl=1)
            # pair is contiguous: xb layout [p, ch, r, bl, m]
            xb = xpool.tile([128, NCH, CH_ROUNDS, 2, 64], db, tag="xb")
            xT2v = xT2[:].rearrange("p (ch bl r) m -> p ch r bl m",
                                    ch=NCH, bl=2)
            nc.gpsimd.tensor_copy(xb[:], xT2v)

            for ch in range(NCH):
                vE = vpool.tile([128, SLOTS, 512], db, tag="vE")
                vO = vpool.tile([128, SLOTS, 512], db, tag="vO")
                # ---- step 1: 2-b stationary [64,128] matmuls, 2-packed ----
                for r in range(CH_ROUNDS):
                    pU0 = ps1.tile([128, 512], dt, tag="U0")
                    pU1 = ps1.tile([128, 512], dt, tag="U1")
                    nc.tensor.matmul(pU0[:], xb[0:64, ch, r], w01[0:64],
                                     start=True, stop=True, tile_position=(0, 0))
                    nc.tensor.matmul(pU1[:], xb[64:128, ch, r], w01[64:128],
                                     start=True, stop=True, tile_position=(64, 0))
                    nc.vector.tensor_copy(vE[:, r], pU0[:])
                    nc.scalar.copy(vO[:, r], pU1[:])

                # ---- step 2 + epilogue, for each half (E: rows 0-127 of
                # super-tile, O: rows 128-255) ----
                for name, v in (("E", vE), ("O", vO)):
                    po = ps2.tile([128, SLOTS, 64], dt, tag="po" + name)
                    v4 = v[:].rearrange("p t (s n) -> p t s n", s=R)
                    for s2 in range(R):
                        nc.tensor.matmul(po[0:64], w23[0:64, s2],
                                         v4[0:64, :, s2],
                                         start=(s2 == 0), stop=(s2 == R - 1),
                                         tile_position=(0, 0))
                        nc.tensor.matmul(po[64:128], w23[64:128, s2],
                                         v4[64:128, :, s2],
                                         start=(s2 == 0), stop=(s2 == R - 1),
                                         tile_position=(64, 64))
                    # bias add (relu commutes with the transpose; applied after)
                    ot = opool.tile([128, SLOTS, 64], db, tag="ot" + name)
                    nc.vector.tensor_tensor(ot[:], po[:], biasq[:],
                                            mybir.AluOpType.add)
                    # ---- transpose back to b-major ----
                    # wide transposes: in [128=(bl,n34), 128=(tp,n12)] ->
                    # out [128=(tp,n12), 128=(bl,n34)]; b = b0 + 8bl + 2c + tp
                    o2 = ot[:].rearrange("p t n -> p (t n)")
                    b0 = st * SUP + (128 if name == "O" else 0) + 16 * ch
                    pt2 = ps3.tile([128, 4, 128], db, tag="pt")
                    for c in range(4):
                        nc.tensor.transpose(pt2[:, c],
                                            o2[:, 128 * c:128 * (c + 1)],
                                            ident[:])
                    # ob stored [p, bl, c, n34]; written via the (c, bl) view
                    ob = opool.tile([128, 2, 4, 64], dt, tag="ob" + name)
                    obw = ob[:].rearrange("p bl c n34 -> p c bl n34")
                    pt2v = pt2[:].rearrange("p c (bl n34) -> p c bl n34", bl=2)
                    nc.scalar.activation(obw, pt2v,
                                         mybir.ActivationFunctionType.Relu)
                    eng = nc.gpsimd if name == "E" else nc.sync
                    q = b0 // 16
                    eng.dma_start(out_v4[q, 0], ob[0:64])
                    eng.dma_start(out_v4[q, 1], ob[64:128])

    nc.compile()
    return nc


def _prep_weights(core0, core1, core2, core3, bias):
    import ml_dtypes

    # w01[(m1,m2),(n1,n2),s2]; einsum dims: m=m1 n=n1 s=s1 / o=m2 p=n2 r=s2
    w01 = np.einsum("mns,sopr->monpr", core0[0], core1).reshape(64, 64, 8)
    # w23[s2,(m3,m4),(n3,n4)]; s m=m3 n=n3 t=s3 / p=m4 q=n4
    w23 = np.einsum("smnt,tpq->smpnq", core2, core3[:, :, :, 0]).reshape(8, 64, 64)

    w01r = w01.transpose(0, 2, 1).reshape(64, 512)        # [m12, (s2 n12)]
    w01r = np.concatenate([w01r, w01r], 0).astype(ml_dtypes.bfloat16)
    w23r = w23.transpose(1, 0, 2)                          # [m34, s2, n34]
    w23r = np.concatenate([w23r, w23r], 0).astype(ml_dtypes.bfloat16)
    bq = bias.reshape(64, 64).T                            # [n34, n12]
    bq = np.broadcast_to(bq[None, :, None, :], (2, 64, SLOTS, 64))
    biasq = np.ascontiguousarray(bq.reshape(128, SLOTS, 64), np.float32)
    return w01r, w23r, biasq


def kernel(x, core0, core1, core2, core3, bias):
    global _compiled
    from concourse.bass_utils import run_bass_kernel_spmd

    if _compiled is None:
        _compiled = _build()
    nc = _compiled

    w01r, w23r, biasq = _prep_weights(
        np.asarray(core0, np.float32), np.asarray(core1, np.float32),
        np.asarray(core2, np.float32), np.asarray(core3, np.float32),
        np.asarray(bias, np.float32))

    x = np.ascontiguousarray(np.asarray(x, np.float32))
    in_maps = [{
        "x": x[c * B:(c + 1) * B],
        "w01": w01r, "w23": w23r, "biasq": biasq,
    } for c in range(NCORES)]
    res = run_bass_kernel_spmd(nc, in_maps, list(range(NCORES)))
    globals()["_last_results"] = res
    out = np.concatenate([res.results[c]["out"] for c in range(NCORES)], axis=0)
    return out.astype(np.float32)

